# revision 23
# baseline (speedup 1.0000x reference)
"""Neural ODE Euler integration on 8 Trainium2 NeuronCores.

h_{n+1} = h_n + 0.1 * tanh(h_n @ W[k].T + b[k]),  k = n // 10,  100 steps.
x: [1024, 1024], W: [10, 1024, 1024], b: [10, 1024].
Returns (features [1024,1024], traj [101,1024,1024]) like the reference.

Strategy: data-parallel over batch (128 rows per core), weights replicated.
The ODE is integrated in u = h/dt scale (dt folded into W on the host) so
the Euler update is a plain add. On-chip state is kept transposed
(uT[i, b], i on partitions in 8 chunks of 128) so each step's matmul uses
uT chunks as the stationary operand and the pre-transposed W as the moving
operand with free dim 512 — the float32r 1-cycle/row tensor-engine path.
The bias enters the same PSUM accumulation as a K=1 ones x bias-row
matmul, so tanh runs as two whole-half [128,512] ACT ops straight from
PSUM into SBUF (normal layout). The tanh halves are then transposed back
on the PE (8 grouped 128x128 transposes, alternating PSUM banks), and per
chunk the vector engine computes the fp32 carry u + t while GPSIMD writes
the fp32r rounded copy that feeds the next step's matmuls.
"""

import contextlib

import numpy as np

import concourse.bacc as bacc
import concourse.mybir as mybir
import concourse.tile as tile
from concourse import bass_utils

F32 = mybir.dt.float32
F32R = mybir.dt.float32r

N_STEPS = 100
NUM_VALS = 10
STEPS_PER_SLAB = N_STEPS // NUM_VALS
DT = 0.1
B, D = 1024, 1024
N_CORES = 8
BL = B // N_CORES  # 128 batch rows per core
NJ = D // 128  # 8 partition chunks of the D axis
HALF = 512  # matmul moving free dim (PSUM bank limit for f32)

_cache: dict = {}


def _build_nc(reps: int = 1):
    """reps>1 wraps the 100-step pass in a hardware loop (timing use only;
    the state is re-initialized from DRAM each pass)."""
    nc = bacc.Bacc("TRN2", target_bir_lowering=False, debug=False)
    xT_d = nc.dram_tensor("xT", [128, D], F32, kind="ExternalInput").ap()
    wT_d = nc.dram_tensor("WT", [NUM_VALS, D, D], F32R, kind="ExternalInput").ap()
    brow_d = nc.dram_tensor("brow", [NUM_VALS, D], F32R, kind="ExternalInput").ap()
    ident_d = nc.dram_tensor("ident", [128, 128], F32, kind="ExternalInput").ap()
    ones_d = nc.dram_tensor("ones", [1, 128], F32R, kind="ExternalInput").ap()
    traj_d = nc.dram_tensor(
        "trajT", [N_STEPS + 1, D, BL], F32, kind="ExternalOutput"
    ).ap()

    with tile.TileContext(nc) as tc:
        with (
            tc.tile_pool(name="const", bufs=1) as constp,
            tc.tile_pool(name="state", bufs=3) as statep,
            tc.tile_pool(name="wslab", bufs=2) as wp,
            tc.tile_pool(name="tsb", bufs=2) as tsbp,
            tc.tile_pool(name="ypsum", bufs=4, space="PSUM") as yp,
            tc.tile_pool(name="tpsum", bufs=4, space="PSUM") as tpp,
        ):
            # bias rows: partition k holds b[k] along the free dim, for the
            # K=1 bias matmul (rhs). ones row is the K=1 stationary operand.
            brow_sb = constp.tile([1, NUM_VALS * D], F32R)
            nc.sync.dma_start(
                brow_sb[:], brow_d[:].rearrange("k o -> (k o)")[None, :]
            )
            ones_sb = constp.tile([1, 128], F32R)
            nc.sync.dma_start(ones_sb[:], ones_d[:])
            ident_sb = constp.tile([128, 128], F32)
            nc.sync.dma_start(ident_sb[:], ident_d[:])

            def load_slab(k):
                t = wp.tile([128, NJ * D], F32R, tag="wslab", name=f"w{k}")
                nc.sync.dma_start(t[:], wT_d[k].rearrange("(i p) o -> p i o", p=128))
                return t

            def load_state():
                uT0 = statep.tile([128, D], F32, tag="state", name="uT_init")
                nc.sync.dma_start(uT0[:], xT_d[:])
                uTr0 = statep.tile([128, D], F32R, tag="stater", name="uTr_init")
                nc.gpsimd.tensor_copy(uTr0[:], uT0[:])
                return uT0, uTr0

            slabs: list = [None] * NUM_VALS
            uT = uTr = None
            if reps == 1:
                slabs[0] = load_slab(0)
                uT, uTr = load_state()
                # traj[0] = x
                nc.sync.dma_start(
                    traj_d[0].rearrange("(j p) b -> p j b", p=128),
                    uT[:].rearrange("p (j b) -> p j b", j=NJ),
                )

            def emit_step(n, uT, uTr, w):
                k = n // STEPS_PER_SLAB
                yps = [
                    yp.tile([128, HALF], F32, tag="y", name=f"y{n}_{h}")
                    for h in range(2)
                ]

                def mm(h, ic):
                    nc.tensor.matmul(
                        yps[h][:],
                        uTr[:, ic * 128 : (ic + 1) * 128],
                        w[:, ic * D + h * HALF : ic * D + (h + 1) * HALF],
                        start=(ic == 0),
                        stop=False,
                    )

                def bias_mm(h):
                    # + b[k] broadcast over the batch: ones[1,128].T @ b-row
                    nc.tensor.matmul(
                        yps[h][:],
                        ones_sb[:],
                        brow_sb[:, k * D + h * HALF : k * D + (h + 1) * HALF],
                        start=False,
                        stop=True,
                    )

                # half h: 8 accumulating matmuls + bias row, then one whole
                # [128,512] tanh from PSUM into SBUF (normal layout)
                t_sb = tsbp.tile([128, D], F32, tag="tsb", name=f"t{n}")
                for h in range(2):
                    for ic in range(NJ):
                        mm(h, ic)
                    bias_mm(h)
                    nc.scalar.activation(
                        t_sb[:, h * HALF : (h + 1) * HALF],
                        yps[h][:],
                        mybir.ActivationFunctionType.Tanh,
                    )

                # grouped PE transposes of the tanh halves (PSUM banks
                # alternate per chunk), then per chunk the fp32 carry on DVE
                # and the fp32r rounded matmul copy on GPSIMD
                uT_new = statep.tile([128, D], F32, tag="state", name=f"uT{n}")
                uTr_new = statep.tile([128, D], F32R, tag="stater", name=f"uTr{n}")
                tps = [
                    tpp.tile([128, HALF], F32, tag="t", name=f"tp{n}_{h}")
                    for h in range(2)
                ]
                for j in range(NJ):
                    nc.tensor.transpose(
                        tps[j % 2][:, (j // 2) * 128 : (j // 2 + 1) * 128],
                        t_sb[:, j * 128 : (j + 1) * 128],
                        ident_sb[:],
                    )
                for j in range(NJ):
                    t_sl = tps[j % 2][:, (j // 2) * 128 : (j // 2 + 1) * 128]
                    sl = slice(j * 128, (j + 1) * 128)
                    nc.vector.tensor_add(uT_new[:, sl], t_sl, uT[:, sl])
                    nc.gpsimd.tensor_copy(uTr_new[:, sl], uT_new[:, sl])

                nc.sync.dma_start(
                    traj_d[n + 1].rearrange("(j p) b -> p j b", p=128),
                    uT_new[:].rearrange("p (j b) -> p j b", j=NJ),
                )
                return uT_new, uTr_new

            def emit_pass(uT, uTr):
                for n in range(N_STEPS):
                    k = n // STEPS_PER_SLAB
                    if n % STEPS_PER_SLAB == 0 and k + 1 < NUM_VALS:
                        slabs[k + 1] = load_slab(k + 1)
                    uT, uTr = emit_step(n, uT, uTr, slabs[k])

            if reps == 1:
                emit_pass(uT, uTr)
            else:
                with tc.For_i(0, reps, 1):
                    slabs[0] = load_slab(0)
                    uT0, uTr0 = load_state()
                    emit_pass(uT0, uTr0)

    nc.compile()
    return nc


def build_in_maps(x, W, b):
    x = np.asarray(x, dtype=np.float32)
    W = np.asarray(W, dtype=np.float32)
    b = np.asarray(b, dtype=np.float32)
    # u = h/DT substitution: u_{n+1} = u_n + tanh((DT*W) u_n + b).
    # WT[k, i, o] = DT * W[k, o, i]
    WT = np.ascontiguousarray(DT * W.transpose(0, 2, 1))
    ident = np.eye(128, dtype=np.float32)
    ones = np.ones((1, 128), dtype=np.float32)

    in_maps = []
    for c in range(N_CORES):
        xs = x[c * BL : (c + 1) * BL]  # [128(b), 1024(i)]
        # xT[p, j*128 + b_l] = x_shard[b_l, j*128+p] / DT
        xT = np.ascontiguousarray(
            (1.0 / DT) * xs.reshape(BL, NJ, 128).transpose(2, 1, 0).reshape(128, D)
        )
        in_maps.append({"xT": xT, "WT": WT, "brow": b, "ident": ident, "ones": ones})
    return in_maps


def kernel(x: np.ndarray, W: np.ndarray, b: np.ndarray):
    if "nc" not in _cache:
        _cache["nc"] = _build_nc()
    nc = _cache["nc"]

    in_maps = build_in_maps(x, W, b)

    res = bass_utils.run_bass_kernel_spmd(nc, in_maps, core_ids=list(range(N_CORES)))
    _cache["last_results"] = res

    traj = np.empty((N_STEPS + 1, B, D), dtype=np.float32)
    for c in range(N_CORES):
        tT = res.results[c]["trajT"]  # [101, 1024(i), 128(b_l)], u = h/DT scale
        traj[:, c * BL : (c + 1) * BL, :] = DT * tT.transpose(0, 2, 1)
    features = traj[N_STEPS].copy()
    return features, traj


# revision 24
# speedup vs baseline: 1.2044x; 1.2044x over previous
"""Neural ODE Euler integration on 8 Trainium2 NeuronCores.

h_{n+1} = h_n + 0.1 * tanh(h_n @ W[k].T + b[k]),  k = n // 10,  100 steps.
x: [1024, 1024], W: [10, 1024, 1024], b: [10, 1024].
Returns (features [1024,1024], traj [101,1024,1024]) like the reference.

Strategy: data-parallel over batch (128 rows per core), weights replicated.
The ODE is integrated in u = h/dt scale (dt folded into W on the host) so
the Euler update is a plain add. On-chip state is kept transposed
(uT[i, b], i on partitions in 8 chunks of 128) so each step's matmul uses
uT chunks as the stationary operand and the pre-transposed W as the moving
operand with free dim 512 — the float32r 1-cycle/row tensor-engine path.
The bias enters the same PSUM accumulation as a K=1 ones x bias-row
matmul, so tanh runs as two whole-half [128,512] ACT ops straight from
PSUM into SBUF (normal layout). The tanh halves are then transposed back
on the PE (8 grouped 128x128 transposes, alternating PSUM banks), and per
chunk the vector engine computes the fp32 carry u + t while GPSIMD writes
the fp32r rounded copy that feeds the next step's matmuls.
"""

import contextlib

import numpy as np

import concourse.bacc as bacc
import concourse.mybir as mybir
import concourse.tile as tile
from concourse import bass_utils

F32 = mybir.dt.float32
F32R = mybir.dt.float32r

N_STEPS = 100
NUM_VALS = 10
STEPS_PER_SLAB = N_STEPS // NUM_VALS
DT = 0.1
B, D = 1024, 1024
N_CORES = 8
BL = B // N_CORES  # 128 batch rows per core
NJ = D // 128  # 8 partition chunks of the D axis
HALF = 512  # matmul moving free dim (PSUM bank limit for f32)

_cache: dict = {}


def _build_nc(reps: int = 1):
    """reps>1 wraps the 100-step pass in a hardware loop (timing use only;
    the state is re-initialized from DRAM each pass)."""
    nc = bacc.Bacc("TRN2", target_bir_lowering=False, debug=False)
    xT_d = nc.dram_tensor("xT", [128, D], F32, kind="ExternalInput").ap()
    wT_d = nc.dram_tensor("WT", [NUM_VALS, D, D], F32R, kind="ExternalInput").ap()
    brow_d = nc.dram_tensor("brow", [NUM_VALS, D], F32R, kind="ExternalInput").ap()
    ident_d = nc.dram_tensor("ident", [128, 128], F32, kind="ExternalInput").ap()
    ones_d = nc.dram_tensor("ones", [1, 128], F32R, kind="ExternalInput").ap()
    traj_d = nc.dram_tensor(
        "trajT", [N_STEPS + 1, D, BL], F32, kind="ExternalOutput"
    ).ap()

    with tile.TileContext(nc) as tc:
        with (
            tc.tile_pool(name="const", bufs=1) as constp,
            tc.tile_pool(name="state", bufs=3) as statep,
            tc.tile_pool(name="wslab", bufs=2) as wp,
            tc.tile_pool(name="tsb", bufs=2) as tsbp,
            tc.tile_pool(name="ypsum", bufs=4, space="PSUM") as yp,
            tc.tile_pool(name="tpsum", bufs=4, space="PSUM") as tpp,
        ):
            # bias rows: partition k holds b[k] along the free dim, for the
            # K=1 bias matmul (rhs). ones row is the K=1 stationary operand.
            brow_sb = constp.tile([1, NUM_VALS * D], F32R)
            nc.sync.dma_start(
                brow_sb[:], brow_d[:].rearrange("k o -> (k o)")[None, :]
            )
            ones_sb = constp.tile([1, 128], F32R)
            nc.sync.dma_start(ones_sb[:], ones_d[:])
            ident_sb = constp.tile([128, 128], F32)
            nc.sync.dma_start(ident_sb[:], ident_d[:])

            def load_slab(k):
                t = wp.tile([128, NJ * D], F32R, tag="wslab", name=f"w{k}")
                nc.sync.dma_start(t[:], wT_d[k].rearrange("(i p) o -> p i o", p=128))
                return t

            def load_state():
                uT0 = statep.tile([128, D], F32, tag="state", name="uT_init")
                nc.sync.dma_start(uT0[:], xT_d[:])
                uTr0 = statep.tile([128, D], F32R, tag="stater", name="uTr_init")
                nc.gpsimd.tensor_copy(uTr0[:], uT0[:])
                return uT0, uTr0

            slabs: list = [None] * NUM_VALS
            uT = uTr = None
            if reps == 1:
                slabs[0] = load_slab(0)
                uT, uTr = load_state()
                # traj[0] = x
                nc.sync.dma_start(
                    traj_d[0].rearrange("(j p) b -> p j b", p=128),
                    uT[:].rearrange("p (j b) -> p j b", j=NJ),
                )

            def emit_step(n, uT, uTr, w):
                k = n // STEPS_PER_SLAB
                yps = [
                    yp.tile([128, HALF], F32, tag="y", name=f"y{n}_{h}")
                    for h in range(2)
                ]

                import os as _os
                _skip_bias = bool(_os.environ.get("SKIP_BIAS"))

                def mm(h, ic):
                    nc.tensor.matmul(
                        yps[h][:],
                        uTr[:, ic * 128 : (ic + 1) * 128],
                        w[:, ic * D + h * HALF : ic * D + (h + 1) * HALF],
                        start=(ic == 0),
                        stop=(_skip_bias and ic == NJ - 1),
                    )

                def bias_mm(h):
                    # + b[k] broadcast over the batch: ones[1,128].T @ b-row
                    nc.tensor.matmul(
                        yps[h][:],
                        ones_sb[:],
                        brow_sb[:, k * D + h * HALF : k * D + (h + 1) * HALF],
                        start=False,
                        stop=True,
                    )

                # half h: 8 accumulating matmuls + bias row, then one whole
                # [128,512] tanh from PSUM into SBUF (normal layout)
                t_sb = tsbp.tile([128, D], F32, tag="tsb", name=f"t{n}")
                import os
                for h in range(2):
                    for ic in range(NJ):
                        mm(h, ic)
                    if not os.environ.get("SKIP_BIAS"):
                        bias_mm(h)
                    nc.scalar.activation(
                        t_sb[:, h * HALF : (h + 1) * HALF],
                        yps[h][:],
                        mybir.ActivationFunctionType.Tanh,
                    )

                # grouped PE transposes of the tanh halves (PSUM banks
                # alternate per chunk), then per chunk the fp32 carry on DVE
                # and the fp32r rounded matmul copy on GPSIMD
                uT_new = statep.tile([128, D], F32, tag="state", name=f"uT{n}")
                uTr_new = statep.tile([128, D], F32R, tag="stater", name=f"uTr{n}")
                tps = [
                    tpp.tile([128, HALF], F32, tag="t", name=f"tp{n}_{h}")
                    for h in range(2)
                ]
                for j in range(NJ):
                    nc.tensor.transpose(
                        tps[j % 2][:, (j // 2) * 128 : (j // 2 + 1) * 128],
                        t_sb[:, j * 128 : (j + 1) * 128],
                        ident_sb[:],
                    )
                for j in range(NJ):
                    t_sl = tps[j % 2][:, (j // 2) * 128 : (j // 2 + 1) * 128]
                    sl = slice(j * 128, (j + 1) * 128)
                    nc.vector.tensor_add(uT_new[:, sl], t_sl, uT[:, sl])
                    nc.gpsimd.tensor_copy(uTr_new[:, sl], uT_new[:, sl])

                nc.sync.dma_start(
                    traj_d[n + 1].rearrange("(j p) b -> p j b", p=128),
                    uT_new[:].rearrange("p (j b) -> p j b", j=NJ),
                )
                return uT_new, uTr_new

            def emit_pass(uT, uTr):
                for n in range(N_STEPS):
                    k = n // STEPS_PER_SLAB
                    if n % STEPS_PER_SLAB == 0 and k + 1 < NUM_VALS:
                        slabs[k + 1] = load_slab(k + 1)
                    uT, uTr = emit_step(n, uT, uTr, slabs[k])

            if reps == 1:
                emit_pass(uT, uTr)
            else:
                with tc.For_i(0, reps, 1):
                    slabs[0] = load_slab(0)
                    uT0, uTr0 = load_state()
                    emit_pass(uT0, uTr0)

    nc.compile()
    return nc


def build_in_maps(x, W, b):
    x = np.asarray(x, dtype=np.float32)
    W = np.asarray(W, dtype=np.float32)
    b = np.asarray(b, dtype=np.float32)
    # u = h/DT substitution: u_{n+1} = u_n + tanh((DT*W) u_n + b).
    # WT[k, i, o] = DT * W[k, o, i]
    WT = np.ascontiguousarray(DT * W.transpose(0, 2, 1))
    ident = np.eye(128, dtype=np.float32)
    ones = np.ones((1, 128), dtype=np.float32)

    in_maps = []
    for c in range(N_CORES):
        xs = x[c * BL : (c + 1) * BL]  # [128(b), 1024(i)]
        # xT[p, j*128 + b_l] = x_shard[b_l, j*128+p] / DT
        xT = np.ascontiguousarray(
            (1.0 / DT) * xs.reshape(BL, NJ, 128).transpose(2, 1, 0).reshape(128, D)
        )
        in_maps.append({"xT": xT, "WT": WT, "brow": b, "ident": ident, "ones": ones})
    return in_maps


def kernel(x: np.ndarray, W: np.ndarray, b: np.ndarray):
    if "nc" not in _cache:
        _cache["nc"] = _build_nc()
    nc = _cache["nc"]

    in_maps = build_in_maps(x, W, b)

    res = bass_utils.run_bass_kernel_spmd(nc, in_maps, core_ids=list(range(N_CORES)))
    _cache["last_results"] = res

    traj = np.empty((N_STEPS + 1, B, D), dtype=np.float32)
    for c in range(N_CORES):
        tT = res.results[c]["trajT"]  # [101, 1024(i), 128(b_l)], u = h/DT scale
        traj[:, c * BL : (c + 1) * BL, :] = DT * tT.transpose(0, 2, 1)
    features = traj[N_STEPS].copy()
    return features, traj


# revision 25
# speedup vs baseline: 1.4385x; 1.1944x over previous
"""Neural ODE Euler integration on 8 Trainium2 NeuronCores.

h_{n+1} = h_n + 0.1 * tanh(h_n @ W[k].T + b[k]),  k = n // 10,  100 steps.
x: [1024, 1024], W: [10, 1024, 1024], b: [10, 1024].
Returns (features [1024,1024], traj [101,1024,1024]) like the reference.

Strategy: data-parallel over batch (128 rows per core), weights replicated.
The ODE is integrated in u = h/dt scale (dt folded into W on the host) so
the Euler update is a plain add. On-chip state is kept transposed
(uT[i, b], i on partitions in 8 chunks of 128) so each step's matmul uses
uT chunks as the stationary operand and the pre-transposed W as the moving
operand with free dim 512 — the float32r 1-cycle/row tensor-engine path.
The carried state stays full fp32; per chunk, tanh+bias runs on the scalar
engine in transposed layout (bias is per-partition there), the vector
engine writes the fp32r rounded state feeding the next step's matmuls, and
GPSIMD writes the fp32 carry. Matmuls and the 8 PE transposes stay in
contiguous groups (interleaving them measurably hurts on hardware), and
each transpose gets its own PSUM bank modulo 4 so an ACT read of chunk j
never blocks the PE write of a later chunk.
"""

import contextlib

import numpy as np

import concourse.bacc as bacc
import concourse.mybir as mybir
import concourse.tile as tile
from concourse import bass_utils

F32 = mybir.dt.float32
F32R = mybir.dt.float32r

N_STEPS = 100
NUM_VALS = 10
STEPS_PER_SLAB = N_STEPS // NUM_VALS
DT = 0.1
B, D = 1024, 1024
N_CORES = 8
BL = B // N_CORES  # 128 batch rows per core
NJ = D // 128  # 8 partition chunks of the D axis
HALF = 512  # matmul moving free dim (PSUM bank limit for f32)

_cache: dict = {}


def _build_nc(reps: int = 1):
    """reps>1 wraps the 100-step pass in a hardware loop (timing use only;
    the state is re-initialized from DRAM each pass)."""
    nc = bacc.Bacc("TRN2", target_bir_lowering=False, debug=False)
    xT_d = nc.dram_tensor("xT", [128, D], F32, kind="ExternalInput").ap()
    wT_d = nc.dram_tensor("WT", [NUM_VALS, D, D], F32R, kind="ExternalInput").ap()
    bias_d = nc.dram_tensor("bias", [128, NUM_VALS * NJ], F32, kind="ExternalInput").ap()
    ident_d = nc.dram_tensor("ident", [128, 128], F32, kind="ExternalInput").ap()
    traj_d = nc.dram_tensor(
        "trajT", [N_STEPS + 1, D, BL], F32, kind="ExternalOutput"
    ).ap()

    with tile.TileContext(nc) as tc:
        with (
            tc.tile_pool(name="const", bufs=1) as constp,
            tc.tile_pool(name="state", bufs=3) as statep,
            tc.tile_pool(name="wslab", bufs=2) as wp,
            tc.tile_pool(name="ysb", bufs=2) as ysbp,
            tc.tile_pool(name="tt", bufs=2) as ttp,
            tc.tile_pool(name="ypsum", bufs=4, space="PSUM") as yp,
            tc.tile_pool(name="tpsum", bufs=4, space="PSUM") as tpp,
        ):
            bias_sb = constp.tile([128, NUM_VALS * NJ], F32)
            nc.sync.dma_start(bias_sb[:], bias_d[:])
            ident_sb = constp.tile([128, 128], F32)
            nc.sync.dma_start(ident_sb[:], ident_d[:])

            def load_slab(k):
                t = wp.tile([128, NJ * D], F32R, tag="wslab", name=f"w{k}")
                nc.sync.dma_start(t[:], wT_d[k].rearrange("(i p) o -> p i o", p=128))
                return t

            def load_state():
                uT0 = statep.tile([128, D], F32, tag="state", name="uT_init")
                nc.sync.dma_start(uT0[:], xT_d[:])
                uTr0 = statep.tile([128, D], F32R, tag="stater", name="uTr_init")
                nc.gpsimd.tensor_copy(uTr0[:], uT0[:])
                return uT0, uTr0

            slabs: list = [None] * NUM_VALS
            uT = uTr = None
            if reps == 1:
                slabs[0] = load_slab(0)
                uT, uTr = load_state()
                # traj[0] = x
                nc.sync.dma_start(
                    traj_d[0].rearrange("(j p) b -> p j b", p=128),
                    uT[:].rearrange("p (j b) -> p j b", j=NJ),
                )

            def emit_step(n, uT, uTr, w):
                k = n // STEPS_PER_SLAB
                yps = [
                    yp.tile([128, HALF], F32, tag="y", name=f"y{n}_{h}")
                    for h in range(2)
                ]

                def mm(h, ic):
                    nc.tensor.matmul(
                        yps[h][:],
                        uTr[:, ic * 128 : (ic + 1) * 128],
                        w[:, ic * D + h * HALF : ic * D + (h + 1) * HALF],
                        start=(ic == 0),
                        stop=(ic == NJ - 1),
                    )

                for ic in range(NJ):
                    mm(0, ic)
                ysb = ysbp.tile([128, D], F32, tag="ysb", name=f"ysb{n}")
                # y half0 PSUM->SBUF quarter-copies overlap half1's matmuls
                nc.vector.tensor_copy(ysb[:, 0:256], yps[0][:, 0:256])
                nc.scalar.copy(ysb[:, 256:512], yps[0][:, 256:512])
                for ic in range(NJ):
                    mm(1, ic)
                nc.vector.tensor_copy(ysb[:, 512:768], yps[1][:, 0:256])
                nc.scalar.copy(ysb[:, 768:1024], yps[1][:, 256:512])

                # 8 grouped PE transposes; chunk j lands in PSUM bank j%4 so
                # tanh reads never block later transpose writes
                uT_new = statep.tile([128, D], F32, tag="state", name=f"uT{n}")
                uTr_new = statep.tile([128, D], F32R, tag="stater", name=f"uTr{n}")
                tt = ttp.tile([128, D], F32, tag="tt", name=f"tt{n}")
                tps = [
                    tpp.tile([128, 256], F32, tag="t", name=f"tp{n}_{q}")
                    for q in range(4)
                ]

                def t_slice(j):
                    return tps[j % 4][:, (j // 4) * 128 : (j // 4 + 1) * 128]

                for j in range(NJ):
                    nc.tensor.transpose(
                        t_slice(j), ysb[:, j * 128 : (j + 1) * 128], ident_sb[:]
                    )
                # per chunk: tanh+bias (ACT, -> SBUF), fp32r state for the
                # next step's matmuls (DVE), fp32 carry (GPSIMD)
                for j in range(NJ):
                    sl = slice(j * 128, (j + 1) * 128)
                    col = k * NJ + j
                    nc.scalar.activation(
                        tt[:, sl],
                        t_slice(j),
                        mybir.ActivationFunctionType.Tanh,
                        bias=bias_sb[:, col : col + 1],
                        scale=1.0,
                    )
                    nc.vector.tensor_add(uTr_new[:, sl], tt[:, sl], uT[:, sl])
                    nc.gpsimd.tensor_add(uT_new[:, sl], tt[:, sl], uT[:, sl])

                nc.sync.dma_start(
                    traj_d[n + 1].rearrange("(j p) b -> p j b", p=128),
                    uT_new[:].rearrange("p (j b) -> p j b", j=NJ),
                )
                return uT_new, uTr_new

            def emit_pass(uT, uTr):
                for n in range(N_STEPS):
                    k = n // STEPS_PER_SLAB
                    if n % STEPS_PER_SLAB == 0 and k + 1 < NUM_VALS:
                        slabs[k + 1] = load_slab(k + 1)
                    uT, uTr = emit_step(n, uT, uTr, slabs[k])

            if reps == 1:
                emit_pass(uT, uTr)
            else:
                with tc.For_i(0, reps, 1):
                    slabs[0] = load_slab(0)
                    uT0, uTr0 = load_state()
                    emit_pass(uT0, uTr0)

    nc.compile()
    return nc


def build_in_maps(x, W, b):
    x = np.asarray(x, dtype=np.float32)
    W = np.asarray(W, dtype=np.float32)
    b = np.asarray(b, dtype=np.float32)
    # u = h/DT substitution: u_{n+1} = u_n + tanh((DT*W) u_n + b).
    # WT[k, i, o] = DT * W[k, o, i]
    WT = np.ascontiguousarray(DT * W.transpose(0, 2, 1))
    # bias_re[p, k*8+j] = b[k, j*128+p]
    bias_re = np.ascontiguousarray(
        b.reshape(NUM_VALS, NJ, 128).transpose(2, 0, 1).reshape(128, NUM_VALS * NJ)
    )
    ident = np.eye(128, dtype=np.float32)

    in_maps = []
    for c in range(N_CORES):
        xs = x[c * BL : (c + 1) * BL]  # [128(b), 1024(i)]
        # xT[p, j*128 + b_l] = x_shard[b_l, j*128+p] / DT
        xT = np.ascontiguousarray(
            (1.0 / DT) * xs.reshape(BL, NJ, 128).transpose(2, 1, 0).reshape(128, D)
        )
        in_maps.append({"xT": xT, "WT": WT, "bias": bias_re, "ident": ident})
    return in_maps


def kernel(x: np.ndarray, W: np.ndarray, b: np.ndarray):
    if "nc" not in _cache:
        _cache["nc"] = _build_nc()
    nc = _cache["nc"]

    in_maps = build_in_maps(x, W, b)

    res = bass_utils.run_bass_kernel_spmd(nc, in_maps, core_ids=list(range(N_CORES)))
    _cache["last_results"] = res

    traj = np.empty((N_STEPS + 1, B, D), dtype=np.float32)
    for c in range(N_CORES):
        tT = res.results[c]["trajT"]  # [101, 1024(i), 128(b_l)], u = h/DT scale
        traj[:, c * BL : (c + 1) * BL, :] = DT * tT.transpose(0, 2, 1)
    features = traj[N_STEPS].copy()
    return features, traj
